# revision 13
# baseline (speedup 1.0000x reference)
"""Trainium2 Bass kernel for nn_CMCI_Mamba.

Strategy: data-parallel over the 2B=8 mamba streams (1 sequence per core).
Each launch runs 2 chained mamba layers fully on-chip in d-major layout
(features on partitions, time on the free axis).

Engine assignment (per layer):
- PE (fp16): in_proj with the causal conv FOLDED IN (4 shifted matmuls with
  host-prescaled weights diag(conv_w_k) @ in_w), z-proj, fused
  dt_w@xp_w[dt] projection, 32 stride-0 B/C broadcast matmuls, out_proj.
- Act: Silu(conv) / Silu(z) straight from PSUM, softplus via Exp+Ln (one
  table set), the 16 per-state dA = exp(A_s * delta) passes, PSUM->SBUF
  fp16 copies of the B/C broadcasts, layer-output copies.
- DVE: the 16 SSM scans (tensor_tensor_scan, batched 2 states per
  instruction with a zeroed dA column resetting the carry), dBu muls,
  half of the hs*C muls/accumulation.
- GPSIMD: the other half of the hs*C muls and y accumulation.

Host does the cheap cross-stream elementwise combines between launches.
"""
import sys
import numpy as np
from contextlib import ExitStack

for _p in ("/opt/trn_rl_repo",):
    if _p not in sys.path:
        sys.path.insert(0, _p)

import concourse.bass as bass
import concourse.bacc as bacc
import concourse.tile as tile
from concourse import mybir
from concourse import bass_utils

T, DM, DI, DS, DR, K, NL = 2048, 64, 128, 16, 4, 4, 2
B, C = 4, 2048
UF = T + K  # padded u width (2052)
FP = mybir.dt.float32
FH = mybir.dt.float16
AX = mybir.AluOpType
AF = mybir.ActivationFunctionType

# fp16 param blob column layout, (128, 1024) per layer
_B_WK = 0       # [0:64, 0:512]    4x conv-scaled in_proj-x lhsT (64,128) each
_B_Z = 512      # [0:64, 512:640]  z lhsT
_B_WD = 640     # [:, 640:768]     (dt_w @ xp_w[:DR]) lhsT
_B_BC = 768     # [:, 768:800]     B/C projection columns (32)
_B_OUT = 800    # [:, 800:864]     out_proj lhsT
_B_OUTD = 864   # [:, 864:928]     out_proj lhsT with D folded (for x*sz term)
_HBLOB_W = 1024
# fp32 blob (128, 20): [:, 0:16]=A (=-exp(A_log)), 16=conv_b, 17=dt_b, 18=D


def _pack_blobs(raw, l):
    hb = np.zeros((DI, _HBLOB_W), np.float16)
    in_w = raw["in_w"][l]          # (256, 64)
    conv_w = raw["conv_w"][l]      # (128, 4)
    for k in range(K):
        wk = in_w[:DI] * conv_w[:, k:k + 1]          # (128, 64)
        hb[:DM, _B_WK + 128 * k:_B_WK + 128 * (k + 1)] = wk.T
    hb[:DM, _B_Z:_B_Z + DI] = in_w[DI:2 * DI].T
    wd = raw["dt_w"][l] @ raw["xp_w"][l][:DR]        # (128, 128)
    hb[:, _B_WD:_B_WD + DI] = wd.T
    hb[:, _B_BC:_B_BC + 2 * DS] = raw["xp_w"][l][DR:DR + 2 * DS].T
    hb[:, _B_OUT:_B_OUT + DM] = raw["out_w"][l].T
    # out_proj with D folded in: out += (out_w * D) @ (x * silu(z))
    hb[:, _B_OUTD:_B_OUTD + DM] = (raw["out_w"][l] * raw["D"][l]).T
    fb = np.zeros((DI, 20), np.float32)
    fb[:, 0:DS] = -np.exp(raw["A_log"][l])
    fb[:, 16] = raw["conv_b"][l]
    fb[:, 17] = raw["dt_b"][l]
    fb[:, 18] = raw["D"][l]
    return hb, fb


def _build_layer(nc, pools, hb, fb, up, upo, out_specs, out_dma):
    """One mamba layer. up/upo: (64, UF) fp16 padded input (+1-shifted copy).
    out_specs: list of (tile, col_off) -- the (64, T) layer output is copied
    (in halves, on Act) into tile[:, off:off+T]. out_dma: DRAM ap or None.
    """
    const, big, sl, ps, gl = pools
    NCH = T // 512
    H = T // 2
    lid = gl["lid"]

    wkT = [hb[0:DM, _B_WK + 128 * k:_B_WK + 128 * (k + 1)] for k in range(K)]
    zT = hb[0:DM, _B_Z:_B_Z + DI]
    wdT = hb[:, _B_WD:_B_WD + DI]
    outT = hb[:, _B_OUT:_B_OUT + DM]
    outDT = hb[:, _B_OUTD:_B_OUTD + DM]
    Acols = fb[:, 0:DS]
    convb = fb[:, 16:17]
    dtb = fb[:, 17:18]

    def bc_mm(tag, col, name):
        """Stride-0 broadcast matmul of projection column `col` -> psum."""
        t = ps.tile([DI, T], FP, tag="bc", name=name)
        w = hb[:, _B_BC + col:_B_BC + col + 1].broadcast_to((DI, DI))
        for c in range(NCH):
            nc.tensor.matmul(t[:, c * 512:(c + 1) * 512], w,
                             xact[:, c * 512:(c + 1) * 512],
                             start=True, stop=True)
        return t

    # ---- in_proj-x with folded causal conv -> silu -> xact (fp16) ----
    # xc[:, t] = sum_k (diag(conv_w_k) @ in_w_x) @ u[:, t-3+k]; tap k reads
    # u_pad[:, c*512+k:]; odd k uses the 1-shifted copy so every rhs offset
    # stays 4B-aligned.  Silu is applied per half so the delta chain starts
    # as soon as the first half lands.
    xact = big.tile([DI, T], FH, tag="xact", name=f"xact{lid}")
    mmx = ps.tile([DI, T], FP, tag="bc", name=f"mmx{lid}")
    for c in range(NCH):
        o = c * 512
        cs = slice(o, o + 512)
        nc.tensor.matmul(mmx[:, cs], wkT[0], up[:, o:o + 512],
                         start=True, stop=False)
        nc.tensor.matmul(mmx[:, cs], wkT[1], upo[:, o:o + 512],
                         start=False, stop=False)
        nc.tensor.matmul(mmx[:, cs], wkT[2], up[:, o + 2:o + 514],
                         start=False, stop=False)
        nc.tensor.matmul(mmx[:, cs], wkT[3], upo[:, o + 2:o + 514],
                         start=False, stop=True)
    for h in range(2):
        hs_ = slice(h * H, (h + 1) * H)
        nc.scalar.activation(xact[:, hs_], mmx[:, hs_], AF.Silu, bias=convb)

    # ---- delta = softplus(dt_proj + dt_b) via Exp then Ln(1+x), halves ----
    delta = big.tile([DI, T], FH, tag="delta", name=f"delta{lid}")
    ev = big.tile([DI, T], FH, tag="ev", name=f"ev{lid}")
    dx = big.tile([DI, T], FH, tag="dx", name=f"dx{lid}")
    mmd = ps.tile([DI, T], FP, tag="bc", name=f"mmd{lid}")
    for c in range(NCH):
        o = c * 512
        nc.tensor.matmul(mmd[:, o:o + 512], wdT, xact[:, o:o + 512],
                         start=True, stop=True)
    for h in range(2):
        hs_ = slice(h * H, (h + 1) * H)
        nc.scalar.activation(ev[:, hs_], mmd[:, hs_], AF.Exp, bias=dtb)
        nc.scalar.activation(delta[:, hs_], ev[:, hs_], AF.Ln, bias=1.0)
        nc.vector.tensor_mul(dx[:, hs_], delta[:, hs_], xact[:, hs_])

    # ---- s-loop: single s=0 (PSUM-direct, shortest ramp), 7 pairs, s=15 ----
    ysn = big.tile([DI, T], FH, tag="ysn", name=f"ysn{lid}")
    yP = big.tile([DI, 2 * T], FH, tag="yP", name=f"yP{lid}")

    # s = 0
    dA0 = big.tile([DI, T], FH, tag="dAs", name=f"dA{lid}_s0")
    for h in range(2):
        hs_ = slice(h * H, (h + 1) * H)
        nc.scalar.activation(dA0[:, hs_], delta[:, hs_], AF.Exp,
                             scale=Acols[:, 0:1])
    bps0 = bc_mm("bc", 0, f"bps{lid}_0")
    dBu0 = big.tile([DI, T], FH, tag="dBus", name=f"dBu{lid}_s0")
    nc.vector.tensor_mul(dBu0[:], dx[:], bps0[:])
    hs0 = big.tile([DI, T], FH, tag="hss", name=f"hs{lid}_s0")
    nc.vector.tensor_tensor_scan(hs0[:], dA0[:], dBu0[:], 0.0, AX.mult, AX.add)
    cps0 = bc_mm("bc", DS + 0, f"cps{lid}_0")
    nc.vector.tensor_mul(ysn[:], hs0[:], cps0[:])

    # pairs (1,2) .. (13,14)
    for p in range(1, 8):
        s0, s1 = 2 * p - 1, 2 * p
        dA = sl.tile([DI, 2 * T], FH, tag="dA", name=f"dA{lid}_{p}")
        nc.scalar.activation(dA[:, 0:T], delta[:], AF.Exp,
                             scale=Acols[:, s0:s0 + 1])
        nc.scalar.activation(dA[:, T:2 * T], delta[:], AF.Exp,
                             scale=Acols[:, s1:s1 + 1])
        # zero the boundary column so the scan carry resets between states
        nc.scalar.activation(dA[:, T:T + 1], gl["zcol"][:], AF.Copy)
        bcrep = sl.tile([DI, 2 * T], FH, tag="bcrep", name=f"brep{lid}_{p}")
        dBu = sl.tile([DI, 2 * T], FH, tag="dBu", name=f"dBu{lid}_{p}")
        for i, s in ((0, s0), (1, s1)):
            bps = bc_mm("bc", s, f"bps{lid}_{s}")
            nc.scalar.activation(bcrep[:, i * T:(i + 1) * T], bps[:], AF.Copy)
            nc.vector.tensor_mul(dBu[:, i * T:(i + 1) * T], dx[:],
                                 bcrep[:, i * T:(i + 1) * T])
        hs = sl.tile([DI, 2 * T], FH, tag="hs", name=f"hs{lid}_{p}")
        nc.vector.tensor_tensor_scan(hs[:], dA[:], dBu[:], 0.0,
                                     AX.mult, AX.add)
        ccrep = sl.tile([DI, 2 * T], FH, tag="ccrep", name=f"crep{lid}_{p}")
        for i, s in ((0, s0), (1, s1)):
            cps = bc_mm("bc", DS + s, f"cps{lid}_{s}")
            nc.scalar.activation(ccrep[:, i * T:(i + 1) * T], cps[:], AF.Copy)
        if p == 1:
            nc.vector.tensor_mul(yP[:], hs[:], ccrep[:])
        else:
            hsc = sl.tile([DI, 2 * T], FH, tag="hsc", name=f"hsc{lid}_{p}")
            nc.vector.tensor_mul(hsc[:], hs[:], ccrep[:])
            nc.vector.tensor_add(yP[:], yP[:], hsc[:])

    # s = 15
    dA15 = big.tile([DI, T], FH, tag="dAs2", name=f"dA{lid}_s15")
    nc.scalar.activation(dA15[:], delta[:], AF.Exp, scale=Acols[:, 15:16])
    bps15 = bc_mm("bc", 15, f"bps{lid}_15")
    brep15 = big.tile([DI, T], FH, tag="dBus2", name=f"brep{lid}_15")
    nc.scalar.activation(brep15[:], bps15[:], AF.Copy)
    dBu15 = big.tile([DI, T], FH, tag="dBuf", name=f"dBu{lid}_15")
    nc.vector.tensor_mul(dBu15[:], dx[:], brep15[:])
    hs15 = big.tile([DI, T], FH, tag="hss2", name=f"hs{lid}_s15")
    nc.vector.tensor_tensor_scan(hs15[:], dA15[:], dBu15[:], 0.0,
                                 AX.mult, AX.add)
    cps15 = bc_mm("bc", DS + 15, f"cps{lid}_15")
    crep15 = big.tile([DI, T], FH, tag="creps", name=f"crep{lid}_15")
    nc.scalar.activation(crep15[:], cps15[:], AF.Copy)
    hsc15 = big.tile([DI, T], FH, tag="hscs", name=f"hsc{lid}_15")
    nc.vector.tensor_mul(hsc15[:], hs15[:], crep15[:])
    nc.vector.tensor_add(ysn[:], ysn[:], hsc15[:])

    # ---- z-proj late (keeps the Act head short; silu set reloads once) ----
    zs = big.tile([DI, T], FH, tag="zs", name=f"zs{lid}")
    mmz = ps.tile([DI, T], FP, tag="bc", name=f"mmz{lid}")
    for c in range(NCH):
        o = c * 512
        nc.tensor.matmul(mmz[:, o:o + 512], zT, upo[:, o + 2:o + 514],
                         start=True, stop=True)
    nc.scalar.activation(zs[:], mmz[:], AF.Silu)
    xsz = big.tile([DI, T], FH, tag="xsz", name=f"xsz{lid}")
    nc.vector.tensor_mul(xsz[:], xact[:], zs[:])

    # ---- y = (sum_s hs*C)*silu(z); out = out_w@y + (out_w*D)@(x*silu(z)) ----
    yf = big.tile([DI, T], FH, tag="yf", name=f"yf{lid}")
    nc.vector.tensor_add(yf[:], yP[:, 0:T], yP[:, T:2 * T])
    nc.vector.tensor_add(yf[:], yf[:], ysn[:])
    nc.vector.tensor_mul(yf[:], yf[:], zs[:])

    mmo = ps.tile([DI, T], FP, tag="bc", name=f"mmo{lid}")
    for c in range(NCH):
        o = c * 512
        nc.tensor.matmul(mmo[0:DM, o:o + 512], outT, yf[:, o:o + 512],
                         start=True, stop=False)
        nc.tensor.matmul(mmo[0:DM, o:o + 512], outDT, xsz[:, o:o + 512],
                         start=False, stop=True)
    # chunked output copies: half h feeds the next layer's half-h head ops
    for h in range(2):
        src = mmo[0:DM, h * H:(h + 1) * H]
        for tl, off in out_specs:
            nc.scalar.activation(tl[:, off + h * H:off + (h + 1) * H],
                                 src, AF.Copy)
        if out_dma is not None:
            nc.sync.dma_start(out_dma[:, h * H:(h + 1) * H],
                              out_specs[0][0][:, out_specs[0][1] + h * H:
                                              out_specs[0][1] + (h + 1) * H])


def _build_kernel(ctx, tc, u0, u0o, hblobs, fblobs, outs):
    nc = tc.nc
    const = ctx.enter_context(tc.tile_pool(name="const", bufs=1))
    big = ctx.enter_context(tc.tile_pool(name="big", bufs=1))
    sl = ctx.enter_context(tc.tile_pool(name="sl", bufs=2))
    ps = ctx.enter_context(tc.tile_pool(name="ps", bufs=2, space="PSUM"))

    hb, fb = [], []
    for l in range(NL):
        t = const.tile([DI, _HBLOB_W], FH, tag=f"hb{l}", name=f"hb{l}")
        nc.sync.dma_start(t[:], hblobs[l][:])
        hb.append(t)
        t = const.tile([DI, 20], FP, tag=f"fb{l}", name=f"fb{l}")
        nc.sync.dma_start(t[:], fblobs[l][:])
        fb.append(t)

    upA = const.tile([DM, UF], FH, tag="upA", name="upA")
    upAo = const.tile([DM, UF], FH, tag="upAo", name="upAo")
    nc.sync.dma_start(upA[:], u0[:])
    nc.sync.dma_start(upAo[:], u0o[:])
    upB = const.tile([DM, UF], FH, tag="upB", name="upB")
    upBo = const.tile([DM, UF], FH, tag="upBo", name="upBo")
    nc.gpsimd.memset(upB[:, 0:K - 1], 0.0)
    nc.gpsimd.memset(upB[:, UF - 1:UF], 0.0)
    nc.gpsimd.memset(upBo[:, 0:K - 2], 0.0)
    nc.gpsimd.memset(upBo[:, UF - 2:UF], 0.0)
    o2 = const.tile([DM, T], FH, tag="o2", name="o2")

    zcol = const.tile([DI, 1], FH, tag="zcol", name="zcol")
    nc.gpsimd.memset(zcol[:], 0.0)

    pools = (const, big, sl, ps, {"lid": 0, "zcol": zcol})
    # layer 1: outputs go to upB[:, 3:3+T] and upBo[:, 2:2+T]
    _build_layer(nc, pools, hb[0], fb[0], upA, upAo,
                 [(upB, K - 1), (upBo, K - 2)], outs[0])
    pools = (const, big, sl, ps, {"lid": 1, "zcol": zcol})
    _build_layer(nc, pools, hb[1], fb[1], upB, upBo, [(o2, 0)], outs[1])


def build_program():
    nc = bacc.Bacc("TRN2", target_bir_lowering=False, debug=False)
    u0 = nc.dram_tensor("u0", [DM, UF], FH, kind="ExternalInput").ap()
    u0o = nc.dram_tensor("u0o", [DM, UF], FH, kind="ExternalInput").ap()
    hblobs = [nc.dram_tensor(f"hblob{l}", [DI, _HBLOB_W], FH,
                             kind="ExternalInput").ap() for l in range(NL)]
    fblobs = [nc.dram_tensor(f"fblob{l}", [DI, 20], FP,
                             kind="ExternalInput").ap() for l in range(NL)]
    outs = [nc.dram_tensor(f"o{l + 1}T", [DM, T], FH,
                           kind="ExternalOutput").ap() for l in range(NL)]
    with tile.TileContext(nc) as tc:
        with ExitStack() as ctx:
            _build_kernel(ctx, tc, u0, u0o, hblobs, fblobs, outs)
    nc.compile()
    return nc


_PROG = None


def _get_prog():
    global _PROG
    if _PROG is None:
        _PROG = build_program()
    return _PROG


def _pad_u(u):
    """u: (64, T) f32 -> (u_pad, u_pad_odd) fp16 (64, UF)."""
    up = np.zeros((DM, UF), np.float16)
    up[:, K - 1:K - 1 + T] = u.astype(np.float16)
    upo = np.zeros((DM, UF), np.float16)
    upo[:, 0:UF - 1] = up[:, 1:UF]
    return up, upo


def _run_launch(u_list_T, raw, trace=False, trace_kwargs=None):
    """u_list_T: list of 8 arrays (64, 2048) f32. raw: param dict (np).
    Returns (o1_list, o2_list, res) with (64, 2048) fp16 outputs."""
    nc = _get_prog()
    blobs = [_pack_blobs(raw, l) for l in range(NL)]
    in_maps = []
    for b in range(8):
        up, upo = _pad_u(np.asarray(u_list_T[b], np.float32))
        in_maps.append({
            "u0": up, "u0o": upo,
            "hblob0": blobs[0][0], "fblob0": blobs[0][1],
            "hblob1": blobs[1][0], "fblob1": blobs[1][1],
        })
    res = bass_utils.run_bass_kernel_spmd(
        nc, in_maps, core_ids=list(range(8)), trace=trace,
        **(trace_kwargs or {}))
    o1 = [res.results[b]["o1T"] for b in range(8)]
    o2 = [res.results[b]["o2T"] for b in range(8)]
    return o1, o2, res


def kernel(**inputs):
    inp = {k: np.asarray(v, np.float32) for k, v in inputs.items()}
    Ms = inp["Ms_feature"]
    Pan = inp["Pan_feature"]
    h = C // 2
    names = ("in_w", "conv_w", "conv_b", "xp_w", "dt_w", "dt_b",
             "A_log", "D", "out_w")
    rawa = {n: inp["a_" + n] for n in names}
    rawb = {n: inp["b_" + n] for n in names}

    cf1 = np.concatenate([Ms[:, :h], Pan[:, h:]], axis=1)
    cf2 = np.concatenate([Pan[:, :h], Ms[:, h:]], axis=1)
    u_list = [cf1[b].T for b in range(B)] + [cf2[b].T for b in range(B)]
    o1, o2, _ = _run_launch(u_list, rawa)
    cf1_1 = np.stack([o1[b].T.astype(np.float32) for b in range(B)])
    cf2_1 = np.stack([o1[B + b].T.astype(np.float32) for b in range(B)])
    cf1_2 = np.stack([o2[b].T.astype(np.float32) for b in range(B)])
    cf2_2 = np.stack([o2[B + b].T.astype(np.float32) for b in range(B)])
    Ms1 = np.maximum((cf1_1 + cf2_1) * 0.5 + Ms, 0.0)
    Ms2 = np.maximum((cf1_2 + cf2_2) * 0.5 + Ms1, 0.0)

    cf3 = np.stack([Pan[:, ::2], Ms2[:, 1::2]], axis=2).reshape(B, C, DM)
    cf4 = np.stack([Ms2[:, ::2], Pan[:, 1::2]], axis=2).reshape(B, C, DM)
    u_list = [cf3[b].T for b in range(B)] + [cf4[b].T for b in range(B)]
    o1, o2, _ = _run_launch(u_list, rawb)
    cf3_1 = np.stack([o1[b].T.astype(np.float32) for b in range(B)])
    cf4_1 = np.stack([o1[B + b].T.astype(np.float32) for b in range(B)])
    cf3_2 = np.stack([o2[b].T.astype(np.float32) for b in range(B)])
    cf4_2 = np.stack([o2[B + b].T.astype(np.float32) for b in range(B)])
    Pan1 = np.maximum((cf3_1 + cf4_1) * 0.5 + Pan, 0.0)
    Pan2 = np.maximum((cf3_2 + cf4_2) * 0.5 + Pan1, 0.0)
    return Ms2, Pan2


# revision 15
# speedup vs baseline: 1.0149x; 1.0149x over previous
"""Trainium2 Bass kernel for nn_CMCI_Mamba.

Strategy: data-parallel over the 2B=8 mamba streams (1 sequence per core).
Each launch runs 2 chained mamba layers fully on-chip in d-major layout
(features on partitions, time on the free axis).

Engine assignment (per layer):
- PE (fp16): in_proj with the causal conv FOLDED IN (4 shifted matmuls with
  host-prescaled weights diag(conv_w_k) @ in_w), z-proj, fused
  dt_w@xp_w[dt] projection, 32 stride-0 B/C broadcast matmuls, out_proj.
- Act: Silu(conv) / Silu(z) straight from PSUM, softplus via Exp+Ln (one
  table set), the 16 per-state dA = exp(A_s * delta) passes, PSUM->SBUF
  fp16 copies of the B/C broadcasts, layer-output copies.
- DVE: the 16 SSM scans (tensor_tensor_scan, batched 2 states per
  instruction with a zeroed dA column resetting the carry), dBu muls,
  half of the hs*C muls/accumulation.
- GPSIMD: the other half of the hs*C muls and y accumulation.

Host does the cheap cross-stream elementwise combines between launches.
"""
import sys
import numpy as np
from contextlib import ExitStack

for _p in ("/opt/trn_rl_repo",):
    if _p not in sys.path:
        sys.path.insert(0, _p)

import concourse.bass as bass
import concourse.bacc as bacc
import concourse.tile as tile
from concourse import mybir
from concourse import bass_utils

T, DM, DI, DS, DR, K, NL = 2048, 64, 128, 16, 4, 4, 2
B, C = 4, 2048
UF = T + K  # padded u width (2052)
FP = mybir.dt.float32
FH = mybir.dt.float16
AX = mybir.AluOpType
AF = mybir.ActivationFunctionType

# fp16 param blob column layout, (128, 1024) per layer
_B_WK = 0       # [0:64, 0:512]    4x conv-scaled in_proj-x lhsT (64,128) each
_B_Z = 512      # [0:64, 512:640]  z lhsT
_B_WD = 640     # [:, 640:768]     (dt_w @ xp_w[:DR]) lhsT
_B_BC = 768     # [:, 768:800]     B/C projection columns (32)
_B_OUT = 800    # [:, 800:864]     out_proj lhsT
_B_OUTD = 864   # [:, 864:928]     out_proj lhsT with D folded (for x*sz term)
_HBLOB_W = 1024
# fp32 blob (128, 20): [:, 0:16]=A (=-exp(A_log)), 16=conv_b, 17=dt_b, 18=D


def _pack_blobs(raw, l):
    hb = np.zeros((DI, _HBLOB_W), np.float16)
    in_w = raw["in_w"][l]          # (256, 64)
    conv_w = raw["conv_w"][l]      # (128, 4)
    for k in range(K):
        wk = in_w[:DI] * conv_w[:, k:k + 1]          # (128, 64)
        hb[:DM, _B_WK + 128 * k:_B_WK + 128 * (k + 1)] = wk.T
    hb[:DM, _B_Z:_B_Z + DI] = in_w[DI:2 * DI].T
    wd = raw["dt_w"][l] @ raw["xp_w"][l][:DR]        # (128, 128)
    hb[:, _B_WD:_B_WD + DI] = wd.T
    hb[:, _B_BC:_B_BC + 2 * DS] = raw["xp_w"][l][DR:DR + 2 * DS].T
    hb[:, _B_OUT:_B_OUT + DM] = raw["out_w"][l].T
    # out_proj with D folded in: out += (out_w * D) @ (x * silu(z))
    hb[:, _B_OUTD:_B_OUTD + DM] = (raw["out_w"][l] * raw["D"][l]).T
    fb = np.zeros((DI, 20), np.float32)
    fb[:, 0:DS] = -np.exp(raw["A_log"][l])
    fb[:, 16] = raw["conv_b"][l]
    fb[:, 17] = raw["dt_b"][l]
    fb[:, 18] = raw["D"][l]
    return hb, fb


def _build_layer(nc, pools, hb, fb, up, upo, out_specs, out_dma):
    """One mamba layer. up/upo: (64, UF) fp16 padded input (+1-shifted copy).
    out_specs: list of (tile, col_off) -- the (64, T) layer output is copied
    (in halves, on Act) into tile[:, off:off+T]. out_dma: DRAM ap or None.
    """
    const, big, sl, ps, gl = pools
    NCH = T // 512
    H = T // 2
    lid = gl["lid"]

    wkT = [hb[0:DM, _B_WK + 128 * k:_B_WK + 128 * (k + 1)] for k in range(K)]
    zT = hb[0:DM, _B_Z:_B_Z + DI]
    wdT = hb[:, _B_WD:_B_WD + DI]
    outT = hb[:, _B_OUT:_B_OUT + DM]
    outDT = hb[:, _B_OUTD:_B_OUTD + DM]
    Acols = fb[:, 0:DS]
    convb = fb[:, 16:17]
    dtb = fb[:, 17:18]

    def bc_mm(tag, col, name):
        """Stride-0 broadcast matmul of projection column `col` -> psum."""
        t = ps.tile([DI, T], FP, tag="bc", name=name)
        w = hb[:, _B_BC + col:_B_BC + col + 1].broadcast_to((DI, DI))
        for c in range(NCH):
            nc.tensor.matmul(t[:, c * 512:(c + 1) * 512], w,
                             xact[:, c * 512:(c + 1) * 512],
                             start=True, stop=True)
        return t

    # ---- in_proj-x with folded causal conv -> silu -> xact (fp16) ----
    # xc[:, t] = sum_k (diag(conv_w_k) @ in_w_x) @ u[:, t-3+k]; tap k reads
    # u_pad[:, c*512+k:]; odd k uses the 1-shifted copy so every rhs offset
    # stays 4B-aligned.  Silu is applied per half so the delta chain starts
    # as soon as the first half lands.
    xact = big.tile([DI, T], FH, tag="xact", name=f"xact{lid}")
    mmx = ps.tile([DI, T], FP, tag="bc", name=f"mmx{lid}")
    for c in range(NCH):
        o = c * 512
        cs = slice(o, o + 512)
        nc.tensor.matmul(mmx[:, cs], wkT[0], up[:, o:o + 512],
                         start=True, stop=False)
        nc.tensor.matmul(mmx[:, cs], wkT[1], upo[:, o:o + 512],
                         start=False, stop=False)
        nc.tensor.matmul(mmx[:, cs], wkT[2], up[:, o + 2:o + 514],
                         start=False, stop=False)
        nc.tensor.matmul(mmx[:, cs], wkT[3], upo[:, o + 2:o + 514],
                         start=False, stop=True)
    for h in range(2):
        hs_ = slice(h * H, (h + 1) * H)
        nc.scalar.activation(xact[:, hs_], mmx[:, hs_], AF.Silu, bias=convb)

    # ---- delta = softplus(dt_proj + dt_b) via Exp then Ln(1+x), halves ----
    delta = big.tile([DI, T], FH, tag="delta", name=f"delta{lid}")
    ev = big.tile([DI, T], FH, tag="ev", name=f"ev{lid}")
    dx = big.tile([DI, T], FH, tag="dx", name=f"dx{lid}")
    mmd = ps.tile([DI, T], FP, tag="bc", name=f"mmd{lid}")
    for c in range(NCH):
        o = c * 512
        nc.tensor.matmul(mmd[:, o:o + 512], wdT, xact[:, o:o + 512],
                         start=True, stop=True)
    for h in range(2):
        hs_ = slice(h * H, (h + 1) * H)
        nc.scalar.activation(ev[:, hs_], mmd[:, hs_], AF.Exp, bias=dtb)
    for h in range(2):
        hs_ = slice(h * H, (h + 1) * H)
        nc.scalar.activation(delta[:, hs_], ev[:, hs_], AF.Ln, bias=1.0)
        nc.vector.tensor_mul(dx[:, hs_], delta[:, hs_], xact[:, hs_])

    # ---- s-loop: single s=0 (PSUM-direct, shortest ramp), 7 pairs, s=15 ----
    ysn = big.tile([DI, T], FH, tag="ysn", name=f"ysn{lid}")
    yP = big.tile([DI, 2 * T], FH, tag="yP", name=f"yP{lid}")

    # s = 0
    dA0 = big.tile([DI, T], FH, tag="dAs", name=f"dA{lid}_s0")
    for h in range(2):
        hs_ = slice(h * H, (h + 1) * H)
        nc.scalar.activation(dA0[:, hs_], delta[:, hs_], AF.Exp,
                             scale=Acols[:, 0:1])
    bps0 = bc_mm("bc", 0, f"bps{lid}_0")
    dBu0 = big.tile([DI, T], FH, tag="dBus", name=f"dBu{lid}_s0")
    nc.vector.tensor_mul(dBu0[:], dx[:], bps0[:])
    hs0 = big.tile([DI, T], FH, tag="hss", name=f"hs{lid}_s0")
    nc.vector.tensor_tensor_scan(hs0[:], dA0[:], dBu0[:], 0.0, AX.mult, AX.add)
    cps0 = bc_mm("bc", DS + 0, f"cps{lid}_0")
    nc.vector.tensor_mul(ysn[:], hs0[:], cps0[:])

    # pairs (1,2) .. (13,14)
    for p in range(1, 8):
        s0, s1 = 2 * p - 1, 2 * p
        dA = sl.tile([DI, 2 * T], FH, tag="dA", name=f"dA{lid}_{p}")
        nc.scalar.activation(dA[:, 0:T], delta[:], AF.Exp,
                             scale=Acols[:, s0:s0 + 1])
        nc.scalar.activation(dA[:, T:2 * T], delta[:], AF.Exp,
                             scale=Acols[:, s1:s1 + 1])
        # zero the boundary column so the scan carry resets between states
        nc.scalar.activation(dA[:, T:T + 1], gl["zcol"][:], AF.Copy)
        bcrep = sl.tile([DI, 2 * T], FH, tag="bcrep", name=f"brep{lid}_{p}")
        dBu = sl.tile([DI, 2 * T], FH, tag="dBu", name=f"dBu{lid}_{p}")
        for i, s in ((0, s0), (1, s1)):
            bps = bc_mm("bc", s, f"bps{lid}_{s}")
            nc.scalar.activation(bcrep[:, i * T:(i + 1) * T], bps[:], AF.Copy)
            nc.vector.tensor_mul(dBu[:, i * T:(i + 1) * T], dx[:],
                                 bcrep[:, i * T:(i + 1) * T])
        hs = sl.tile([DI, 2 * T], FH, tag="hs", name=f"hs{lid}_{p}")
        nc.vector.tensor_tensor_scan(hs[:], dA[:], dBu[:], 0.0,
                                     AX.mult, AX.add)
        ccrep = sl.tile([DI, 2 * T], FH, tag="ccrep", name=f"crep{lid}_{p}")
        for i, s in ((0, s0), (1, s1)):
            cps = bc_mm("bc", DS + s, f"cps{lid}_{s}")
            nc.scalar.activation(ccrep[:, i * T:(i + 1) * T], cps[:], AF.Copy)
        if p == 1:
            nc.vector.tensor_mul(yP[:], hs[:], ccrep[:])
        else:
            hsc = sl.tile([DI, 2 * T], FH, tag="hsc", name=f"hsc{lid}_{p}")
            nc.vector.tensor_mul(hsc[:], hs[:], ccrep[:])
            nc.vector.tensor_add(yP[:], yP[:], hsc[:])

    # s = 15
    dA15 = big.tile([DI, T], FH, tag="dAs2", name=f"dA{lid}_s15")
    nc.scalar.activation(dA15[:], delta[:], AF.Exp, scale=Acols[:, 15:16])
    bps15 = bc_mm("bc", 15, f"bps{lid}_15")
    brep15 = big.tile([DI, T], FH, tag="dBus2", name=f"brep{lid}_15")
    nc.scalar.activation(brep15[:], bps15[:], AF.Copy)
    dBu15 = big.tile([DI, T], FH, tag="dBuf", name=f"dBu{lid}_15")
    nc.vector.tensor_mul(dBu15[:], dx[:], brep15[:])
    hs15 = big.tile([DI, T], FH, tag="hss2", name=f"hs{lid}_s15")
    nc.vector.tensor_tensor_scan(hs15[:], dA15[:], dBu15[:], 0.0,
                                 AX.mult, AX.add)
    cps15 = bc_mm("bc", DS + 15, f"cps{lid}_15")
    crep15 = big.tile([DI, T], FH, tag="creps", name=f"crep{lid}_15")
    nc.scalar.activation(crep15[:], cps15[:], AF.Copy)
    hsc15 = big.tile([DI, T], FH, tag="hscs", name=f"hsc{lid}_15")
    nc.vector.tensor_mul(hsc15[:], hs15[:], crep15[:])
    nc.vector.tensor_add(ysn[:], ysn[:], hsc15[:])

    # ---- z-proj late (keeps the Act head short; silu set reloads once) ----
    zs = big.tile([DI, T], FH, tag="zs", name=f"zs{lid}")
    mmz = ps.tile([DI, T], FP, tag="bc", name=f"mmz{lid}")
    for c in range(NCH):
        o = c * 512
        nc.tensor.matmul(mmz[:, o:o + 512], zT, upo[:, o + 2:o + 514],
                         start=True, stop=True)
    nc.scalar.activation(zs[:], mmz[:], AF.Silu)
    xsz = big.tile([DI, T], FH, tag="xsz", name=f"xsz{lid}")
    nc.vector.tensor_mul(xsz[:], xact[:], zs[:])

    # ---- y = (sum_s hs*C)*silu(z); out = out_w@y + (out_w*D)@(x*silu(z)) ----
    yf = big.tile([DI, T], FH, tag="yf", name=f"yf{lid}")
    nc.vector.tensor_add(yf[:], yP[:, 0:T], yP[:, T:2 * T])
    nc.vector.tensor_add(yf[:], yf[:], ysn[:])
    nc.vector.tensor_mul(yf[:], yf[:], zs[:])

    mmo = ps.tile([DI, T], FP, tag="bc", name=f"mmo{lid}")
    for c in range(NCH):
        o = c * 512
        nc.tensor.matmul(mmo[0:DM, o:o + 512], outT, yf[:, o:o + 512],
                         start=True, stop=False)
        nc.tensor.matmul(mmo[0:DM, o:o + 512], outDT, xsz[:, o:o + 512],
                         start=False, stop=True)
    # chunked output copies: half h feeds the next layer's half-h head ops
    for h in range(2):
        src = mmo[0:DM, h * H:(h + 1) * H]
        for tl, off in out_specs:
            nc.scalar.activation(tl[:, off + h * H:off + (h + 1) * H],
                                 src, AF.Copy)
        if out_dma is not None:
            nc.sync.dma_start(out_dma[:, h * H:(h + 1) * H],
                              out_specs[0][0][:, out_specs[0][1] + h * H:
                                              out_specs[0][1] + (h + 1) * H])


def _build_kernel(ctx, tc, u0, u0o, hblobs, fblobs, outs):
    nc = tc.nc
    const = ctx.enter_context(tc.tile_pool(name="const", bufs=1))
    big = ctx.enter_context(tc.tile_pool(name="big", bufs=1))
    sl = ctx.enter_context(tc.tile_pool(name="sl", bufs=2))
    ps = ctx.enter_context(tc.tile_pool(name="ps", bufs=2, space="PSUM"))

    hb, fb = [], []
    for l in range(NL):
        t = const.tile([DI, _HBLOB_W], FH, tag=f"hb{l}", name=f"hb{l}")
        nc.sync.dma_start(t[:], hblobs[l][:])
        hb.append(t)
        t = const.tile([DI, 20], FP, tag=f"fb{l}", name=f"fb{l}")
        nc.sync.dma_start(t[:], fblobs[l][:])
        fb.append(t)

    upA = const.tile([DM, UF], FH, tag="upA", name="upA")
    upAo = const.tile([DM, UF], FH, tag="upAo", name="upAo")
    nc.sync.dma_start(upA[:], u0[:])
    nc.sync.dma_start(upAo[:], u0o[:])
    upB = const.tile([DM, UF], FH, tag="upB", name="upB")
    upBo = const.tile([DM, UF], FH, tag="upBo", name="upBo")
    nc.gpsimd.memset(upB[:, 0:K - 1], 0.0)
    nc.gpsimd.memset(upB[:, UF - 1:UF], 0.0)
    nc.gpsimd.memset(upBo[:, 0:K - 2], 0.0)
    nc.gpsimd.memset(upBo[:, UF - 2:UF], 0.0)
    o2 = const.tile([DM, T], FH, tag="o2", name="o2")

    # PE warm-up: ~4us of dummy matmuls while input DMAs land, so the HAM
    # clock gate is already at 8/8 when the real in_proj matmuls start.
    wz = const.tile([DI, 512], FH, tag="wz", name="wz")
    nc.gpsimd.memset(wz[:], 0.0)
    wps = ps.tile([DI, T], FP, tag="bc", name="warm")
    for i in range(10):
        nc.tensor.matmul(wps[:, 0:512], wz[:, 0:DI], wz[:],
                         start=True, stop=True)

    zcol = const.tile([DI, 1], FH, tag="zcol", name="zcol")
    nc.gpsimd.memset(zcol[:], 0.0)

    pools = (const, big, sl, ps, {"lid": 0, "zcol": zcol})
    # layer 1: outputs go to upB[:, 3:3+T] and upBo[:, 2:2+T]
    _build_layer(nc, pools, hb[0], fb[0], upA, upAo,
                 [(upB, K - 1), (upBo, K - 2)], outs[0])
    pools = (const, big, sl, ps, {"lid": 1, "zcol": zcol})
    _build_layer(nc, pools, hb[1], fb[1], upB, upBo, [(o2, 0)], outs[1])


def build_program():
    nc = bacc.Bacc("TRN2", target_bir_lowering=False, debug=False)
    u0 = nc.dram_tensor("u0", [DM, UF], FH, kind="ExternalInput").ap()
    u0o = nc.dram_tensor("u0o", [DM, UF], FH, kind="ExternalInput").ap()
    hblobs = [nc.dram_tensor(f"hblob{l}", [DI, _HBLOB_W], FH,
                             kind="ExternalInput").ap() for l in range(NL)]
    fblobs = [nc.dram_tensor(f"fblob{l}", [DI, 20], FP,
                             kind="ExternalInput").ap() for l in range(NL)]
    outs = [nc.dram_tensor(f"o{l + 1}T", [DM, T], FH,
                           kind="ExternalOutput").ap() for l in range(NL)]
    with tile.TileContext(nc) as tc:
        with ExitStack() as ctx:
            _build_kernel(ctx, tc, u0, u0o, hblobs, fblobs, outs)
    nc.compile()
    return nc


_PROG = None


def _get_prog():
    global _PROG
    if _PROG is None:
        _PROG = build_program()
    return _PROG


def _pad_u(u):
    """u: (64, T) f32 -> (u_pad, u_pad_odd) fp16 (64, UF)."""
    up = np.zeros((DM, UF), np.float16)
    up[:, K - 1:K - 1 + T] = u.astype(np.float16)
    upo = np.zeros((DM, UF), np.float16)
    upo[:, 0:UF - 1] = up[:, 1:UF]
    return up, upo


def _run_launch(u_list_T, raw, trace=False, trace_kwargs=None):
    """u_list_T: list of 8 arrays (64, 2048) f32. raw: param dict (np).
    Returns (o1_list, o2_list, res) with (64, 2048) fp16 outputs."""
    nc = _get_prog()
    blobs = [_pack_blobs(raw, l) for l in range(NL)]
    in_maps = []
    for b in range(8):
        up, upo = _pad_u(np.asarray(u_list_T[b], np.float32))
        in_maps.append({
            "u0": up, "u0o": upo,
            "hblob0": blobs[0][0], "fblob0": blobs[0][1],
            "hblob1": blobs[1][0], "fblob1": blobs[1][1],
        })
    res = bass_utils.run_bass_kernel_spmd(
        nc, in_maps, core_ids=list(range(8)), trace=trace,
        **(trace_kwargs or {}))
    o1 = [res.results[b]["o1T"] for b in range(8)]
    o2 = [res.results[b]["o2T"] for b in range(8)]
    return o1, o2, res


def kernel(**inputs):
    inp = {k: np.asarray(v, np.float32) for k, v in inputs.items()}
    Ms = inp["Ms_feature"]
    Pan = inp["Pan_feature"]
    h = C // 2
    names = ("in_w", "conv_w", "conv_b", "xp_w", "dt_w", "dt_b",
             "A_log", "D", "out_w")
    rawa = {n: inp["a_" + n] for n in names}
    rawb = {n: inp["b_" + n] for n in names}

    cf1 = np.concatenate([Ms[:, :h], Pan[:, h:]], axis=1)
    cf2 = np.concatenate([Pan[:, :h], Ms[:, h:]], axis=1)
    u_list = [cf1[b].T for b in range(B)] + [cf2[b].T for b in range(B)]
    o1, o2, _ = _run_launch(u_list, rawa)
    cf1_1 = np.stack([o1[b].T.astype(np.float32) for b in range(B)])
    cf2_1 = np.stack([o1[B + b].T.astype(np.float32) for b in range(B)])
    cf1_2 = np.stack([o2[b].T.astype(np.float32) for b in range(B)])
    cf2_2 = np.stack([o2[B + b].T.astype(np.float32) for b in range(B)])
    Ms1 = np.maximum((cf1_1 + cf2_1) * 0.5 + Ms, 0.0)
    Ms2 = np.maximum((cf1_2 + cf2_2) * 0.5 + Ms1, 0.0)

    cf3 = np.stack([Pan[:, ::2], Ms2[:, 1::2]], axis=2).reshape(B, C, DM)
    cf4 = np.stack([Ms2[:, ::2], Pan[:, 1::2]], axis=2).reshape(B, C, DM)
    u_list = [cf3[b].T for b in range(B)] + [cf4[b].T for b in range(B)]
    o1, o2, _ = _run_launch(u_list, rawb)
    cf3_1 = np.stack([o1[b].T.astype(np.float32) for b in range(B)])
    cf4_1 = np.stack([o1[B + b].T.astype(np.float32) for b in range(B)])
    cf3_2 = np.stack([o2[b].T.astype(np.float32) for b in range(B)])
    cf4_2 = np.stack([o2[B + b].T.astype(np.float32) for b in range(B)])
    Pan1 = np.maximum((cf3_1 + cf4_1) * 0.5 + Pan, 0.0)
    Pan2 = np.maximum((cf3_2 + cf4_2) * 0.5 + Pan1, 0.0)
    return Ms2, Pan2
